# revision 9
# baseline (speedup 1.0000x reference)
"""DPS perturbed-top-k patch-extraction kernel for Trainium2 (Bass/Tile).

Contract: kernel(**inputs) takes the FULL inputs
    x_high  (8, 3, 512, 512) f32
    scores_2d (8, 16, 16) f32
    noise   (8, 500, 256) f32
and returns the FULL output (128, 3, 64, 64) f32.

Sharding: pure data-parallel over batch b across the 8 NeuronCores (one
image per core). No collectives.

Per-core algorithm (matches the reference bit-closely):
  1. min-max normalize scores  s = (sc - min) * recip(max - min + 1e-5)
  2. pert[n,d] = s[d] + 0.05*noise[n,d]     (500, 256)
  3. top-16 threshold per row via vector max8 -> match_replace -> max8
     (exact: verified no fp32 ties at the rank-16/17 boundary for this input)
  4. A = pert >= t written into an 18-stride embedded axis d' = 18*i + j
     (324 wide); cnt = cumsum(A) along d' via tensor_tensor_scan
  5. S'_k[d'] = sum_n f_k(cnt) via transpose + per-k accumulate;
     G_k = S'_k - S'_{k+1};  ind_k[d'] = (G_k[d'] - G_k[d'-1]) / 500
     (any per-k d'-constant offset cancels in the d'-difference, which lets
     ACT use relu(cnt-k) and DVE use max(cnt,k) interchangeably)
  6. out[k] = sum_{i,j} ind[k,18i+j] * patch(i,j) computed as a single
     18x18-block matmul: out_q[(q,k), (c,h',w')] = IND^T @ B with
     B[(a,b), (c,h',w')] = x_pad 32x32 blocks (no overlap redundancy) and
     IND the indicator tile shifted by (18*hq + wq) partitions per output
     quadrant q. f32r matmul (1 cyc/row) with optional fp32 fallback.
"""
import os
import numpy as np
from contextlib import ExitStack

# ---- problem constants (hardcoded per spec) ----
NB = 8           # batch / cores
C = 3
H = W = 512
GS = 16          # score grid 16x16
D2 = 256         # compact d
GE = 18          # embedded grid stride
D3 = GE * GE     # 324
K = 16
N = 500
NCH = 4          # n chunks
NP = 125         # rows per chunk
CM = 108         # partitions per block-chunk (6*18)
PATCH = 64
BLK = 32         # block size (stride between patches)
SIG = 0.05
INV_N = 1.0 / 500.0
NEG = -1.0e30
FREE_B = C * BLK * BLK   # 3072 floats per block partition
NSL = 6                  # 512-wide free slices of FREE_B
O_ROW = C * PATCH * PATCH  # 12288 floats per output patch

_CACHE = {}


def _build_nc():
    import concourse.bacc as bacc
    import concourse.bass as bass
    import concourse.mybir as mybir
    import concourse.tile as tile

    F32 = mybir.dt.float32
    F32R = mybir.dt.float32r
    BF16 = mybir.dt.bfloat16
    I32 = mybir.dt.int32
    ALU = mybir.AluOpType
    ACTF = mybir.ActivationFunctionType
    AP = bass.AP

    use_f32r = os.environ.get("DPS_FP32_MM", "0") != "1"
    MMT = F32R if use_f32r else F32

    nc = bacc.Bacc("TRN2", target_bir_lowering=False, debug=False)
    x_d = nc.dram_tensor("x", (C, H, W), F32, kind="ExternalInput")
    sc_d = nc.dram_tensor("sc", (GS, GS), F32, kind="ExternalInput")
    nz_d = nc.dram_tensor("nz", (N, D2), F32, kind="ExternalInput")
    o_d = nc.dram_tensor("o", (K, O_ROW), F32, kind="ExternalOutput")

    with tile.TileContext(nc) as tc, ExitStack() as ctx:
        sb = ctx.enter_context(tc.tile_pool(name="sb", bufs=1))
        ps_rep = ctx.enter_context(tc.tile_pool(name="ps_rep", bufs=1, space="PSUM"))
        ps_cnt = ctx.enter_context(tc.tile_pool(name="ps_cnt", bufs=1, space="PSUM"))
        ps_out = ctx.enter_context(tc.tile_pool(name="ps_out", bufs=3, space="PSUM"))

        def ap_of(t, off_elems, dims):
            return AP(t.tensor, t[:].offset + off_elems, dims)

        # ---------------- B tiles: 18x18 grid of 32x32 blocks ----------------
        B = [sb.tile([CM, FREE_B], F32, tag=f"B{m}", name=f"B{m}") for m in range(3)]
        for m in range(3):
            nc.gpsimd.memset(B[m][:], 0.0)
        # Per block-row a (0..16), load both w-halves with b running along
        # partitions (stride 1) so every DMA footprint is canonical:
        #   w-left : w' in 16..31 of blocks b=0..15  <- x cols C = 32b + w'-16
        #   w-right: w' in  0..15 of blocks b=1..16  <- x cols C = 32(b-1)+16+w'
        # Rows a=0 / a=16 only have the valid h'-half; a=17 stays zero.
        for a in range(0, 17):
            m, a2 = divmod(a, 6)
            if a == 0:
                h0, r0, nh = 16, 0, 16
            elif a == 16:
                h0, r0, nh = 0, 496, 16
            else:
                h0, r0, nh = 0, BLK * a - 16, 32
            for c in range(C):
                dst = ap_of(B[m], (GE * a2) * FREE_B + c * 1024
                            + h0 * BLK + 16,
                            [[FREE_B, 16], [BLK, nh], [1, 16]])
                src = AP(x_d, c * H * W + r0 * W + 0,
                         [[BLK, 16], [W, nh], [1, 16]])
                nc.sync.dma_start(dst, src)
                dst = ap_of(B[m], (GE * a2 + 1) * FREE_B + c * 1024
                            + h0 * BLK,
                            [[FREE_B, 16], [BLK, nh], [1, 16]])
                src = AP(x_d, c * H * W + r0 * W + 16,
                         [[BLK, 16], [W, nh], [1, 16]])
                nc.sync.dma_start(dst, src)

        # ---------------- scores normalization ----------------
        s256 = sb.tile([1, D2], F32)
        nc.sync.dma_start(s256[:], sc_d[:].rearrange("a b -> (a b)").unsqueeze(0))
        smax = sb.tile([1, 1], F32)
        smin = sb.tile([1, 1], F32)
        nc.vector.tensor_reduce(smax[:], s256[:], axis=mybir.AxisListType.X,
                                op=ALU.max)
        nc.vector.tensor_reduce(smin[:], s256[:], axis=mybir.AxisListType.X,
                                op=ALU.min)
        Dt = sb.tile([1, 1], F32)
        nc.vector.tensor_scalar(Dt[:], smax[:], smin[:], 1e-5,
                                op0=ALU.subtract, op1=ALU.add)
        rD = sb.tile([1, 1], F32)
        nc.vector.reciprocal(rD[:], Dt[:])
        s_row = sb.tile([1, D2], F32)
        nc.vector.tensor_scalar(s_row[:], s256[:], smin[:], rD[:],
                                op0=ALU.subtract, op1=ALU.mult)

        ones = sb.tile([1, 128], F32)
        nc.vector.memset(ones[:], 1.0)

        # ---------------- identity + bias tables ----------------
        iota_t = sb.tile([128, 128], I32)
        nc.gpsimd.iota(iota_t[:], pattern=[[-1, 128]], base=0,
                       channel_multiplier=1)
        ident = sb.tile([128, 128], BF16)
        nc.vector.tensor_scalar(ident[:], iota_t[:], 0, None, op0=ALU.is_equal)
        # 0.05 * identity (fp32) for the pert-by-matmul trick
        diag05 = sb.tile([128, 128], F32)
        nc.vector.tensor_scalar(diag05[:], iota_t[:], 0, SIG,
                                op0=ALU.is_equal, op1=ALU.mult)
        bias_i = sb.tile([128, 17], I32)
        nc.gpsimd.iota(bias_i[:], pattern=[[-1, 17]], base=0,
                       channel_multiplier=0)
        bias_f = sb.tile([128, 17], F32)
        nc.vector.tensor_copy(bias_f[:], bias_i[:])

        # ---------------- per-chunk top-k threshold + cnt ----------------
        cnt = [sb.tile([128, D3], BF16, tag=f"cnt{t}", name=f"cnt{t}") for t in range(NCH)]
        cntT = [ps_cnt.tile([CM, 512], BF16, tag=f"cntT{m}", name=f"cntT{m}") for m in range(3)]
        for t in range(NCH):
            nz_t = sb.tile([128, D2], F32, tag=f"nz{t}", name=f"nzt{t}")
            nc.sync.dma_start(nz_t[0:NP, :], nz_d[NP * t:NP * (t + 1), :])
            pert_ps = ps_rep.tile([128, D2], F32, tag="pert_ps",
                                  name=f"pert_ps{t}", bufs=2)
            nc.tensor.matmul(pert_ps[0:NP, :], ones[:, 0:NP], s_row[:],
                             start=True, stop=False)
            nc.tensor.matmul(pert_ps[0:NP, :], diag05[0:NP, 0:NP],
                             nz_t[0:NP, :], start=False, stop=True)
            pert = sb.tile([128, D2], F32, tag=f"pert{t}", name=f"pert{t}")
            nc.scalar.copy(pert[0:NP, :], pert_ps[0:NP, :])
            top8 = sb.tile([128, 8], F32, tag=f"top8{t}", name=f"top8_{t}")
            nc.vector.max(top8[0:NP, :], pert[0:NP, :])
            pert2 = sb.tile([128, D2], F32, tag=f"pert2{t}", name=f"pert2_{t}")
            nc.vector.match_replace(pert2[0:NP, :], top8[0:NP, :],
                                    pert[0:NP, :], NEG)
            top8b = sb.tile([128, 8], F32, tag=f"top8b{t}", name=f"top8b_{t}")
            nc.vector.max(top8b[0:NP, :], pert2[0:NP, :])

            A = sb.tile([128, D3], F32, tag=f"A{t}", name=f"A{t}")
            nc.gpsimd.memset(A[:], 0.0)
            # strided write of the compact 256 into the 18-stride embedding
            a_view = ap_of(A, 0, [[D3, NP], [GE, GS], [1, GS]])
            p_view = ap_of(pert, 0, [[D2, NP], [GS, GS], [1, GS]])
            nc.vector.tensor_scalar(a_view, p_view, top8b[0:NP, 7:8], None,
                                    op0=ALU.is_ge)
            nc.vector.memset(cnt[t][:], 0.0)
            nc.vector.tensor_tensor_scan(cnt[t][0:NP, :], A[0:NP, :],
                                         A[0:NP, :], initial=0.0,
                                         op0=ALU.add, op1=ALU.bypass)
        # transposes into PSUM (d' on partitions)
        for t in range(NCH):
            for m in range(3):
                nc.tensor.transpose(
                    cntT[m][:, 128 * t:128 * (t + 1)],
                    cnt[t][:, CM * m:CM * (m + 1)], ident[:])

        # ---------------- S' accumulations ----------------
        Sp = [sb.tile([CM, 17], F32, tag=f"Sp{m}", name=f"Sp{m}") for m in range(3)]
        scr_a = sb.tile([CM, 512], BF16, tag="scr_a", name="scr_a")
        scr_v = sb.tile([CM, 512], BF16, tag="scr_v", name="scr_v")
        for m in range(3):
            nc.vector.memset(Sp[m][:, 16:17], 0.0)
            for k in range(16):
                if k < 13:
                    nc.scalar.activation(
                        scr_a[:], cntT[m][:], ACTF.Relu,
                        bias=bias_f[0:CM, k:k + 1], scale=1.0,
                        accum_out=Sp[m][:, k:k + 1])
                else:
                    nc.vector.tensor_scalar(
                        scr_v[:], cntT[m][:], float(k), None,
                        op0=ALU.max, op1=ALU.add,
                        accum_out=Sp[m][:, k:k + 1])

        # ---------------- G, shift, indicators ----------------
        Gc = [sb.tile([CM, K], F32, tag=f"Gc{m}", name=f"Gc{m}") for m in range(3)]
        Gsh = [sb.tile([CM, K], F32, tag=f"Gsh{m}", name=f"Gsh{m}") for m in range(3)]
        ind = [sb.tile([CM, K], F32, tag=f"ind{m}", name=f"ind{m}") for m in range(3)]
        for m in range(3):
            g = sb.tile([CM, K], F32, tag=f"G{m}", name=f"G{m}")
            nc.vector.tensor_tensor(g[:], Sp[m][:, 0:16], Sp[m][:, 1:17],
                                    op=ALU.subtract)
            nc.vector.tensor_scalar_mul(Gc[m][:], g[:], INV_N)
        # Row d'=0 has no predecessor: its "G[-1]" must carry the same per-k
        # d'-constant offsets the mixed relu/max accumulation forms imply
        # (ACT relu-form: 0; DVE max-form S'_k = S_k + 512k).
        def _cform(k):
            return 0.0 if (k < 13 or k == 16) else 512.0 * k
        offs = [(_cform(k) - _cform(k + 1)) * INV_N for k in range(16)]
        k0 = 0
        while k0 < 16:
            k1 = k0
            while k1 < 16 and offs[k1] == offs[k0]:
                k1 += 1
            nc.vector.memset(Gsh[0][0:1, k0:k1], offs[k0])
            k0 = k1
        for m in range(3):
            nc.sync.dma_start(Gsh[m][1:CM, :], Gc[m][0:CM - 1, :])
            if m > 0:
                nc.sync.dma_start(Gsh[m][0:1, :], Gc[m - 1][CM - 1:CM, :])
        for m in range(3):
            nc.vector.tensor_tensor(ind[m][:], Gc[m][:], Gsh[m][:],
                                    op=ALU.subtract)

        # IND: 4 quadrant columns = partition-shifted copies of ind
        IND = [sb.tile([CM, 64], F32, tag=f"IND{m}", name=f"IND{m}") for m in range(3)]
        for m in range(3):
            nc.vector.memset(IND[m][:], 0.0)
        for hq in range(2):
            for wq in range(2):
                q = 2 * hq + wq
                d = GE * hq + wq
                for m in range(3):
                    if d == 0:
                        nc.sync.dma_start(IND[m][:, 16 * q:16 * (q + 1)],
                                          ind[m][:])
                    else:
                        nc.sync.dma_start(
                            IND[m][d:CM, 16 * q:16 * (q + 1)],
                            ind[m][0:CM - d, :])
                        if m > 0:
                            nc.sync.dma_start(
                                IND[m][0:d, 16 * q:16 * (q + 1)],
                                ind[m - 1][CM - d:CM, :])

        # ---------------- main matmul + output ----------------
        # rounding copies to f32r for the 1-cyc/row PE path (walrus requires
        # f32r matmul operands to be produced as f32r)
        if use_f32r:
            Br = [sb.tile([CM, FREE_B], F32R, tag=f"Br{m}", name=f"Br{m}")
                  for m in range(3)]
            nc.vector.tensor_copy(Br[0][:], B[0][:])
            nc.scalar.copy(Br[1][:], B[1][:])
            nc.scalar.copy(Br[2][:], B[2][:])
            INDr = [sb.tile([CM, 64], F32R, tag=f"INDr{m}", name=f"INDr{m}")
                    for m in range(3)]
            for m in range(3):
                nc.vector.tensor_copy(INDr[m][:], IND[m][:])
        else:
            Br, INDr = B, IND
        for t in range(NSL):
            mm = ps_out.tile([64, 512], F32, tag="mm", name=f"mm{t}")
            for m in range(3):
                nc.tensor.matmul(mm[:], INDr[m][:],
                                 Br[m][:, 512 * t:512 * (t + 1)],
                                 start=(m == 0), stop=(m == 2))
            osb = sb.tile([64, 512], F32, tag="osb", name=f"osb{t}")
            if t % 2 == 0:
                nc.scalar.copy(osb[:], mm[:])
            else:
                nc.vector.tensor_copy(osb[:], mm[:])
            c, hh = divmod(t, 2)
            for hq in range(2):
                for wq in range(2):
                    q = 2 * hq + wq
                    dst = AP(o_d, c * 4096 + hh * 1024 + hq * 2048 + wq * 32,
                             [[O_ROW, K], [PATCH, 16], [1, BLK]])
                    src = ap_of(osb, (16 * q) * 512,
                                [[512, K], [BLK, 16], [1, BLK]])
                    nc.sync.dma_start(dst, src)

    nc.compile()
    return nc


def _get_nc():
    if "nc" not in _CACHE:
        _CACHE["nc"] = _build_nc()
    return _CACHE["nc"]


def _run(x_high, scores_2d, noise, trace=False):
    from concourse import bass_utils
    nc = _get_nc()
    x_high = np.ascontiguousarray(x_high, dtype=np.float32)
    scores_2d = np.ascontiguousarray(scores_2d, dtype=np.float32)
    noise = np.ascontiguousarray(noise, dtype=np.float32)
    in_maps = [
        {"x": x_high[i], "sc": scores_2d[i], "nz": noise[i]}
        for i in range(NB)
    ]
    res = bass_utils.run_bass_kernel_spmd(
        nc, in_maps, core_ids=list(range(NB)), trace=trace)
    out = np.concatenate(
        [res.results[i]["o"].reshape(K, C, PATCH, PATCH) for i in range(NB)],
        axis=0)
    return out, res


def kernel(x_high, scores_2d, noise):
    out, _ = _run(x_high, scores_2d, noise, trace=False)
    return out
